# revision 12
# baseline (speedup 1.0000x reference)
"""GCN (6-layer GCNConv) Trainium2 Bass kernel — v3.

Data-parallel over batch (1 mesh per NeuronCore). Per layer
out = A_hat @ (x @ W) + b with A_hat = D^-1/2 (A+I) D^-1/2 shared across batch
and layers.

v3 changes vs v2 (trace-driven):
  - The symmetric norm factorizes: norm_e = dinv[src]*dinv[dst]. Tables store
    h' = dinv (.) (x W); the one-hot segment-sum matrices become 0/1 and are
    precomputed on the HOST and uploaded once (bf16, exact). The dst-side dinv
    is fused into the PSUM evacuation via scalar.activation(scale=AP).
    Kills the per-(tile,layer) DVE is_eq/mult one-hot builds (~1.0 ms).
  - Self-loop: identity matmul accumulated into the same PSUM group (h'_t
    contributes dinv_i h_i; with the outer dinv_i scale this is dinv_i^2 h_i).
    Bias (zero in this model) would go in as a rank-1 K=1 matmul.
  - Messages/tables/weights in bf16: fp32 matmuls stream at half rate
    (539 ns vs ~220 ns for N=512) and double the gather bytes.
  - ONE dma_gather per (tile, layer) fetches all C*128 edge rows, replacing
    C indirect_dma_start calls (994 ns fixed SWDGE cost each; the 2916
    indirect DMAs were 4.2 ms of serialized GpSimd time in v2).
  - Layer 5/6 tables stay f32 (64-wide bf16 rows would violate dma_gather's
    256B row-size minimum); their messages are DVE-cast to bf16 for the PE.
"""
import sys
import time

sys.path.insert(0, "/opt/trn_rl_repo")
import numpy as np
import ml_dtypes
from contextlib import ExitStack

import concourse.bass as bass
import concourse.mybir as mybir
import concourse.tile as tile
from concourse import library_config
from concourse.bass_utils import run_bass_kernel_spmd
from concourse.masks import make_identity

P = 128
F32 = mybir.dt.float32
BF16 = mybir.dt.bfloat16
I16 = mybir.dt.int16
BF = ml_dtypes.bfloat16

_msw_ctr = [0]


def _split_multiwaits(nc, max_waits=1):
    """This walrus build rejects >1 sync wait per instruction: split extras
    onto preceding same-engine NOPs."""
    for f in nc.m.functions:
        for b in f.blocks:
            out, changed = [], False
            for inst in b.instructions:
                si = getattr(inst, "sync_info", None)
                waits = list(si.on_wait) if si is not None else []
                if len(waits) > max_waits:
                    changed = True
                    for w in waits[:-max_waits]:
                        _msw_ctr[0] += 1
                        nop = mybir.InstNoOp(name=f"msw-{_msw_ctr[0]}", ins=[], outs=[])
                        nop.engine = inst.engine
                        nop.sync_info = mybir.SyncInfo(on_wait=[w], on_update=[])
                        out.append(nop)
                    si.on_wait = waits[-max_waits:]
                out.append(inst)
            if changed:
                b.instructions = out
    return nc


def _pack_graph(src, dst, N):
    """Relabel nodes into degree-balanced 128-node tiles (no self-loops in the
    edge list). Returns raw per-tile edge tables."""
    T = (N + P - 1) // P
    NP = T * P
    indeg = np.bincount(dst, minlength=N)          # no-loop in-degree
    C = max(1, int(np.ceil(len(src) / (T * P))))

    order = np.argsort(-indeg, kind="stable")
    while True:
        cap = C * P
        load = np.zeros(T, np.int64)
        count = np.zeros(T, np.int64)
        assign = np.empty(N, np.int64)
        ok = True
        for v in order:
            d = int(indeg[v])
            best_t, best_rem = -1, -1
            for t in range(T):
                if count[t] < P:
                    rem = cap - load[t]
                    if rem > best_rem:
                        best_rem, best_t = rem, t
            if best_t < 0 or load[best_t] + d > cap:
                ok = False
                break
            assign[v] = best_t
            load[best_t] += d
            count[best_t] += 1
        if ok:
            break
        C += 1

    perm = np.full(NP, -1, np.int64)
    new_of_old = np.empty(N, np.int64)
    cursor = np.zeros(T, np.int64)
    for v in range(N):
        t = assign[v]
        nid = t * P + cursor[t]
        cursor[t] += 1
        perm[nid] = v
        new_of_old[v] = nid

    # symmetric normalization (degree INCLUDES self-loops, per GCN)
    deg = (indeg + 1).astype(np.float32)
    dinv = (1.0 / np.sqrt(deg, dtype=np.float32)).astype(np.float32)

    src_n = new_of_old[src]
    dst_n = new_of_old[dst]
    tile_of_e = dst_n // P
    order_e = np.argsort(tile_of_e, kind="stable")
    src_n, dst_n = src_n[order_e], dst_n[order_e]
    tile_of_e = tile_of_e[order_e]

    gsrc = np.zeros((T, C, P), np.int32)
    slot = np.zeros((T, C, P), np.int32)
    ne = np.zeros(T, np.int64)
    starts = np.searchsorted(tile_of_e, np.arange(T + 1))
    for t in range(T):
        lo, hi = starts[t], starts[t + 1]
        n_e = hi - lo
        assert n_e <= C * P, (t, n_e, C * P)
        ne[t] = n_e
        fs = np.zeros(C * P, np.int32)
        fl = np.zeros(C * P, np.int32)
        fs[:n_e] = src_n[lo:hi]
        fl[:n_e] = (dst_n[lo:hi] - t * P)
        gsrc[t] = fs.reshape(C, P)
        slot[t] = fl.reshape(C, P)

    dinv_new = np.zeros(NP, np.float32)
    valid = perm >= 0
    dinv_new[valid] = dinv[perm[valid]]

    return dict(NP=NP, T=T, C=C, perm=perm, gsrc=gsrc, slot=slot, ne=ne,
                dinv_new=dinv_new)


def _build_nc(NP, T, C, FM, F5, FO, has_bias):
    nc = bass.Bass(dynamic_dma_scratch_size=32768)
    KM = FM // P
    IW = C * P // 16          # idx columns per tile (wrapped into 16 rows)
    Ident = mybir.ActivationFunctionType.Identity
    Relu = mybir.ActivationFunctionType.Relu

    d = {}
    d["xT1"] = nc.dram_tensor("xT1", [3, NP], BF16, kind="ExternalInput")
    d["hc1"] = nc.dram_tensor("hc1", [1, FM], BF16, kind="ExternalInput")
    d["W1v"] = nc.dram_tensor("W1v", [3, FM], BF16, kind="ExternalInput")
    for i in (2, 3, 4):
        d[f"W{i}"] = nc.dram_tensor(f"W{i}", [FM, FM], BF16, kind="ExternalInput")
    d["W5"] = nc.dram_tensor("W5", [FM, P], BF16, kind="ExternalInput")
    d["W6"] = nc.dram_tensor("W6", [P, FO], BF16, kind="ExternalInput")
    d["oh01"] = nc.dram_tensor("oh01", [P, T * C * P], BF16, kind="ExternalInput")
    d["idx16"] = nc.dram_tensor("idx16", [P, T * IW], I16, kind="ExternalInput")
    d["dinvc"] = nc.dram_tensor("dinvc", [P, T], F32, kind="ExternalInput")
    d["dinv2c"] = nc.dram_tensor("dinv2c", [P, T], F32, kind="ExternalInput")
    if has_bias:
        d["invd"] = nc.dram_tensor("invd", [1, NP], BF16, kind="ExternalInput")
        d["brows"] = nc.dram_tensor("brows", [1, 4 * FM + P + FO], BF16,
                                    kind="ExternalInput")
    out_d = nc.dram_tensor("out", [NP, FO], F32, kind="ExternalOutput")

    h512 = [nc.dram_tensor(f"h{i}", [NP, FM], BF16, kind="Internal")
            for i in (1, 2, 3, 4)]
    h5_d = nc.dram_tensor("h5", [NP, P], BF16, kind="Internal")
    x6_d = nc.dram_tensor("x6", [NP, P], BF16, kind="Internal")

    with tile.TileContext(nc) as tc:
        with ExitStack() as ctx:
            res = ctx.enter_context(tc.tile_pool(name="res", bufs=1))
            nc.gpsimd.load_library(library_config.mlp)
            oh_sb = res.tile([P, T * C * P], BF16)
            nc.sync.dma_start(out=oh_sb[:], in_=d["oh01"][:, :])
            idx_sb = res.tile([P, T * IW], I16)
            nc.sync.dma_start(out=idx_sb[:], in_=d["idx16"][:, :])
            dinvc_sb = res.tile([P, T], F32)
            nc.sync.dma_start(out=dinvc_sb[:], in_=d["dinvc"][:, :])
            dinv2c_sb = res.tile([P, T], F32)
            nc.sync.dma_start(out=dinv2c_sb[:], in_=d["dinv2c"][:, :])
            ident_bf = res.tile([P, P], BF16)
            make_identity(nc, ident_bf[:])
            ident_f = res.tile([P, P], F32)
            make_identity(nc, ident_f[:])
            ones1 = res.tile([1, P], BF16)
            nc.gpsimd.memset(ones1[:], 1.0)
            nreg = nc.gpsimd.to_reg(2 * C * P)
            nreg1 = nc.gpsimd.to_reg(C * P)
            if has_bias:
                invd_sb = res.tile([1, NP], BF16)
                nc.sync.dma_start(out=invd_sb[:], in_=d["invd"][:, :])
                brows_sb = res.tile([1, 4 * FM + F5 + FO], BF16)
                nc.sync.dma_start(out=brows_sb[:], in_=d["brows"][:, :])

            def oh(t, c):
                s = (t * C + c) * P
                return oh_sb[:, s:s + P]

            # ---- layer 1 dense: h'1 = dinv (.) (x W1 + 1 hc^T) ----
            with tc.tile_pool(name="l1", bufs=1) as l1p, \
                 tc.tile_pool(name="l1ps", bufs=2, space="PSUM") as l1ps, \
                 tc.tile_pool(name="l1sb", bufs=3) as l1sb:
                xT1_sb = l1p.tile([3, NP], BF16)
                nc.sync.dma_start(out=xT1_sb[:], in_=d["xT1"][:, :])
                W1v_sb = l1p.tile([3, FM], BF16)
                nc.sync.dma_start(out=W1v_sb[:], in_=d["W1v"][:, :])
                hc1_sb = l1p.tile([1, FM], BF16)
                nc.sync.dma_start(out=hc1_sb[:], in_=d["hc1"][:, :])
                for t in range(T):
                    ph = l1ps.tile([P, FM], F32, tag="ph")
                    nc.tensor.matmul(out=ph[:], lhsT=xT1_sb[:, t * P:(t + 1) * P],
                                     rhs=W1v_sb[:], start=True, stop=False)
                    nc.tensor.matmul(out=ph[:], lhsT=ones1[:], rhs=hc1_sb[:],
                                     start=False, stop=True)
                    hs = l1sb.tile([P, FM], BF16, tag="hs")
                    nc.scalar.activation(out=hs[:], in_=ph[:], func=Ident,
                                         scale=dinvc_sb[:, t:t + 1])
                    nc.sync.dma_start(out=h512[0][t * P:(t + 1) * P, :], in_=hs[:])

            # ---- merged phases: A(layer i) + dense(i+1), i = 1..4 ----
            for i in (1, 2, 3, 4):
                relu = i in (2, 4)
                h_src = h512[i - 1]
                F_out = FM if i < 4 else P
                h_dst = h512[i] if i < 4 else h5_d
                dst_dt = BF16
                W_d = d[f"W{i + 1}"]
                with tc.tile_pool(name=f"ph{i}", bufs=3) as sp, \
                     tc.tile_pool(name=f"ph{i}m", bufs=2) as smp, \
                     tc.tile_pool(name=f"ph{i}w", bufs=1) as wp, \
                     tc.tile_pool(name=f"ph{i}ps", bufs=2, space="PSUM") as pp, \
                     tc.tile_pool(name=f"ph{i}pt", bufs=2, space="PSUM") as pt, \
                     tc.tile_pool(name=f"ph{i}pd", bufs=2, space="PSUM") as pd:
                    W_sb = [wp.tile([P, F_out], BF16, tag=f"w{k}", name=f"w{i}_{k}")
                            for k in range(KM)]
                    for k in range(KM):
                        nc.sync.dma_start(out=W_sb[k][:], in_=W_d[k * P:(k + 1) * P, :])
                    msgs_of = {}
                    for t in range(T):
                        if t % 1 == 0:
                            nt = min(1, T - t)
                            msgs = smp.tile([P, nt * C * FM], BF16, tag="msg",
                                            name=f"m{i}_{t}")
                            nc.gpsimd.dma_gather(
                                out_ap=msgs[:].rearrange("p (c f) -> p c f",
                                                         c=nt * C),
                                in_ap=h_src[:, :],
                                idxs_ap=idx_sb[:, t * IW:(t + nt) * IW],
                                num_idxs=nt * C * P,
                                num_idxs_reg=nreg if nt == 2 else nreg1,
                                elem_size=FM,
                            )
                            msgs_of[t] = (msgs, 0)
                            if nt == 2:
                                msgs_of[t + 1] = (msgs, C)
                        msgs, c0 = msgs_of.pop(t)
                        hre = sp.tile([P, FM], BF16, tag="hre", name=f"hre{i}_{t}")
                        nc.sync.dma_start(out=hre[:], in_=h_src[t * P:(t + 1) * P, :])
                        pa = pp.tile([P, FM], F32, tag="pa", name=f"pa{i}_{t}")
                        for c in range(C):
                            nc.tensor.matmul(
                                out=pa[:], lhsT=oh(t, c),
                                rhs=msgs[:, (c0 + c) * FM:(c0 + c + 1) * FM],
                                start=(c == 0), stop=False)
                        nc.tensor.matmul(out=pa[:], lhsT=ident_bf[:], rhs=hre[:],
                                         start=False, stop=not has_bias)
                        if has_bias:
                            nc.tensor.matmul(
                                out=pa[:],
                                lhsT=invd_sb[:, t * P:(t + 1) * P],
                                rhs=brows_sb[:, (i - 1) * FM:i * FM],
                                start=False, stop=True)
                        node = sp.tile([P, FM], BF16, tag="node", name=f"nd{i}_{t}")
                        nc.scalar.activation(out=node[:], in_=pa[:],
                                             func=Relu if relu else Ident,
                                             scale=dinvc_sb[:, t:t + 1])
                        ptr = pt.tile([P, FM], BF16, tag="ptr", name=f"pt{i}_{t}")
                        stage = sp.tile([P, FM], BF16, tag="stage", name=f"st{i}_{t}")
                        for fo in range(KM):
                            nc.tensor.matmul(
                                out=ptr[:, fo * P:(fo + 1) * P],
                                lhsT=node[:, fo * P:(fo + 1) * P],
                                rhs=ident_bf[:], is_transpose=True,
                                start=True, stop=True)
                            nc.vector.tensor_copy(
                                out=stage[:, fo * P:(fo + 1) * P],
                                in_=ptr[:, fo * P:(fo + 1) * P])
                        pdt = pd.tile([P, F_out], F32, tag="pd", name=f"pd{i}_{t}")
                        for k in range(KM):
                            nc.tensor.matmul(out=pdt[:], lhsT=stage[:, k * P:(k + 1) * P],
                                             rhs=W_sb[k][:], start=(k == 0),
                                             stop=(k == KM - 1))
                        hs = sp.tile([P, F_out], dst_dt, tag="hs", name=f"hs{i}_{t}")
                        nc.vector.tensor_scalar_mul(
                            out=hs[:], in0=pdt[:],
                            scalar1=dinvc_sb[:, t:t + 1])
                        nc.sync.dma_start(out=h_dst[t * P:(t + 1) * P, :], in_=hs[:])

            # ---- phase 5: A(layer 5) -> x6' = dinv^2 (.) pa5 (128-wide pad) ----
            with tc.tile_pool(name="s5", bufs=3) as sp5, \
                 tc.tile_pool(name="s5ps", bufs=2, space="PSUM") as pp5:
                m5_of = {}
                for t in range(T):
                    if t % 1 == 0:
                        nt = min(1, T - t)
                        m5 = sp5.tile([P, nt * C * P], BF16, tag="m5",
                                      name=f"m5_{t}")
                        nc.gpsimd.dma_gather(
                            out_ap=m5[:].rearrange("p (c f) -> p c f", c=nt * C),
                            in_ap=h5_d[:, :],
                            idxs_ap=idx_sb[:, t * IW:(t + nt) * IW],
                            num_idxs=nt * C * P,
                            num_idxs_reg=nreg if nt == 2 else nreg1,
                            elem_size=P,
                        )
                        m5_of[t] = (m5, 0)
                        if nt == 2:
                            m5_of[t + 1] = (m5, C)
                    m5, c0 = m5_of.pop(t)
                    h5t = sp5.tile([P, P], BF16, tag="h5t", name=f"h5t_{t}")
                    nc.sync.dma_start(out=h5t[:], in_=h5_d[t * P:(t + 1) * P, :])
                    pa5 = pp5.tile([P, P], F32, tag="pa5", name=f"pa5_{t}")
                    for c in range(C):
                        nc.tensor.matmul(out=pa5[:], lhsT=oh(t, c),
                                         rhs=m5[:, (c0 + c) * P:(c0 + c + 1) * P],
                                         start=(c == 0), stop=False)
                    nc.tensor.matmul(out=pa5[:], lhsT=ident_bf[:], rhs=h5t[:],
                                     start=False, stop=not has_bias)
                    if has_bias:
                        nc.tensor.matmul(
                            out=pa5[:],
                            lhsT=invd_sb[:, t * P:(t + 1) * P],
                            rhs=brows_sb[:, 4 * FM:4 * FM + P],
                            start=False, stop=True)
                    x6s = sp5.tile([P, P], BF16, tag="x6s", name=f"x6s_{t}")
                    nc.scalar.activation(out=x6s[:], in_=pa5[:], func=Ident,
                                         scale=dinv2c_sb[:, t:t + 1])
                    nc.sync.dma_start(out=x6_d[t * P:(t + 1) * P, :], in_=x6s[:])

            # ---- phase 6: A(layer 6) feature-major + dense W6 + out ----
            with tc.tile_pool(name="s6", bufs=3) as sp6, \
                 tc.tile_pool(name="s6w", bufs=1) as wp6, \
                 tc.tile_pool(name="s6ps", bufs=2, space="PSUM") as pp6, \
                 tc.tile_pool(name="s6pd", bufs=2, space="PSUM") as pd6:
                W6_sb = wp6.tile([P, FO], BF16)
                nc.sync.dma_start(out=W6_sb[:], in_=d["W6"][:, :])
                m6_of = {}
                for t in range(T):
                    if t % 1 == 0:
                        nt = min(1, T - t)
                        m6 = sp6.tile([P, nt * C * P], BF16, tag="m6",
                                      name=f"m6_{t}")
                        nc.gpsimd.dma_gather(
                            out_ap=m6[:].rearrange("p (c f) -> p c f", c=nt * C),
                            in_ap=x6_d[:, :],
                            idxs_ap=idx_sb[:, t * IW:(t + nt) * IW],
                            num_idxs=nt * C * P,
                            num_idxs_reg=nreg if nt == 2 else nreg1,
                            elem_size=P,
                        )
                        m6_of[t] = (m6, 0)
                        if nt == 2:
                            m6_of[t + 1] = (m6, C)
                    m6, c0 = m6_of.pop(t)
                    x6t = sp6.tile([P, P], BF16, tag="x6t", name=f"x6t_{t}")
                    nc.sync.dma_start(out=x6t[:], in_=x6_d[t * P:(t + 1) * P, :])
                    pg = pp6.tile([P, P], F32, tag="pg", name=f"pg_{t}")
                    nc.tensor.matmul(out=pg[:], lhsT=x6t[:], rhs=ident_bf[:],
                                     start=True, stop=False)
                    for c in range(C):
                        nc.tensor.matmul(
                            out=pg[:], lhsT=m6[:, (c0 + c) * P:(c0 + c + 1) * P],
                            rhs=oh(t, c),
                            start=False, stop=(c == C - 1))
                    gst = sp6.tile([P, P], BF16, tag="gst", name=f"g_{t}")
                    nc.vector.tensor_copy(out=gst[:], in_=pg[:])
                    pf = pd6.tile([P, FO], F32, tag="pf", name=f"pf_{t}")
                    nc.tensor.matmul(out=pf[:], lhsT=gst[:], rhs=W6_sb[:],
                                     start=True, stop=not has_bias)
                    if has_bias:
                        nc.tensor.matmul(
                            out=pf[:],
                            lhsT=invd_sb[:, t * P:(t + 1) * P],
                            rhs=brows_sb[:, 4 * FM + P:],
                            start=False, stop=True)
                    osb = sp6.tile([P, FO], F32, tag="os", name=f"o_{t}")
                    nc.scalar.activation(out=osb[:], in_=pf[:], func=Ident,
                                         scale=dinvc_sb[:, t:t + 1])
                    nc.sync.dma_start(out=out_d[t * P:(t + 1) * P, :], in_=osb[:])

    _postprocess(nc)
    return nc


def _postprocess(nc):
    """HW lowering: split multi-waits (this walrus rejects >1/instruction) and
    encode ISA-subclass instructions (library reload, dma_gather) to bytes.
    simcheck.py stubs this out — CoreSim interprets the typed forms."""
    _split_multiwaits(nc)
    mybir.codegen_inst_isa_subclasses(nc)
    return nc


def _prepare(batch_vertices, img_features, edge_indices,
             W1, b1, W2, b2, W3, b3, W4, b4, W5, b5, W6, b6):
    B, N, _ = batch_vertices.shape
    FM = W1.shape[1]
    F5 = W5.shape[1]
    FO = W6.shape[1]

    ei = np.asarray(edge_indices).astype(np.int64)
    g = _pack_graph(ei[0], ei[1], N)
    NP, T, C, perm = g["NP"], g["T"], g["C"], g["perm"]
    gsrc, slot, ne, dinv_new = g["gsrc"], g["slot"], g["ne"], g["dinv_new"]

    # host one-hot (0/1, dummy edge slots masked)
    j = np.arange(P, dtype=np.int32)
    oh = (slot[..., None] == j).astype(np.float32)          # [T, C, P, 128]
    eslot = (np.arange(C * P).reshape(C, P))
    valid_e = (eslot[None, :, :] < ne[:, None, None])       # [T, C, P]
    oh *= valid_e[..., None]
    oh_dev = np.ascontiguousarray(
        oh.transpose(2, 0, 1, 3).reshape(P, T * C * P)).astype(BF)

    # wrapped int16 gather indices: idx i of tile t at [i%16, t*IW + i//16]
    IW = C * P // 16
    flat = gsrc.reshape(T, C * P).astype(np.int16)          # c-major per tile
    wrapped = flat.reshape(T, IW, 16).transpose(0, 2, 1)    # [T, 16, IW]
    wrapped = np.tile(wrapped, (1, 8, 1))                   # [T, 128, IW]
    idx16 = np.ascontiguousarray(wrapped.transpose(1, 0, 2).reshape(P, T * IW))

    dinvc = np.ascontiguousarray(dinv_new.reshape(T, P).T)
    dinv2c = np.ascontiguousarray((dinv_new ** 2).reshape(T, P).T)

    biases = [np.asarray(b, np.float32) for b in (b2, b3, b4, b5, b6)]
    has_bias = any(np.abs(b).max() > 0 for b in biases)

    hc = (img_features.astype(np.float32) @ W1[3:].astype(np.float32)
          + b1.astype(np.float32)[None, :])

    valid = perm >= 0
    vperm = np.zeros((B, NP, 3), np.float32)
    vperm[:, valid, :] = batch_vertices[:, perm[valid], :]

    common = {
        "W1v": np.ascontiguousarray(W1[:3]).astype(BF),
        "W2": np.ascontiguousarray(W2).astype(BF),
        "W3": np.ascontiguousarray(W3).astype(BF),
        "W4": np.ascontiguousarray(W4).astype(BF),
        "W5": np.ascontiguousarray(
            np.pad(np.asarray(W5, np.float32), ((0, 0), (0, 128 - W5.shape[1])))
        ).astype(BF),
        "W6": np.ascontiguousarray(
            np.pad(np.asarray(W6, np.float32), ((0, 128 - W6.shape[0]), (0, 0)))
        ).astype(BF),
        "oh01": oh_dev, "idx16": idx16,
        "dinvc": dinvc, "dinv2c": dinv2c,
    }
    if has_bias:
        invd = np.zeros(NP, np.float32)
        invd[valid] = 1.0 / dinv_new[valid]
        common["invd"] = invd[None, :].astype(BF)
        bpad = biases[:3] + [np.pad(biases[3], (0, 128 - biases[3].size)),
                             biases[4]]
        common["brows"] = np.concatenate(bpad)[None, :].astype(BF)
    in_maps = []
    for b in range(B):
        m = dict(common)
        m["xT1"] = np.ascontiguousarray(vperm[b].T).astype(BF)
        m["hc1"] = hc[b][None, :].astype(BF)
        in_maps.append(m)
    meta = dict(NP=NP, T=T, C=C, perm=perm, valid=valid, B=B, N=N,
                FM=FM, F5=F5, FO=FO, has_bias=has_bias)
    return in_maps, meta


_BUILD_CACHE = {}


def run(inputs, trace=False):
    in_maps, meta = _prepare(**inputs)
    key = (meta["NP"], meta["C"], meta["FM"], meta["F5"], meta["FO"],
           meta["has_bias"])
    if key not in _BUILD_CACHE:
        t0 = time.time()
        _BUILD_CACHE[key] = _build_nc(meta["NP"], meta["T"], meta["C"],
                                      meta["FM"], meta["F5"], meta["FO"],
                                      meta["has_bias"])
        print(f"[kernel] built bass program in {time.time()-t0:.1f}s", file=sys.stderr)
    nc = _BUILD_CACHE[key]
    B = meta["B"]
    res = run_bass_kernel_spmd(nc, in_maps, core_ids=list(range(B)), trace=trace)
    perm, valid, N = meta["perm"], meta["valid"], meta["N"]
    out = np.empty((B, N, meta["FO"]), np.float32)
    for b in range(B):
        dev = res.results[b]["out"]
        out[b, perm[valid], :] = dev[valid, :]
    return out, res


def kernel(**inputs) -> np.ndarray:
    out, _ = run(inputs)
    return out


# revision 13
# speedup vs baseline: 1.1152x; 1.1152x over previous
"""GCN (6-layer GCNConv) Trainium2 Bass kernel — v3.

Data-parallel over batch (1 mesh per NeuronCore). Per layer
out = A_hat @ (x @ W) + b with A_hat = D^-1/2 (A+I) D^-1/2 shared across batch
and layers.

v3 changes vs v2 (trace-driven):
  - The symmetric norm factorizes: norm_e = dinv[src]*dinv[dst]. Tables store
    h' = dinv (.) (x W); the one-hot segment-sum matrices become 0/1 and are
    precomputed on the HOST and uploaded once (bf16, exact). The dst-side dinv
    is fused into the PSUM evacuation via scalar.activation(scale=AP).
    Kills the per-(tile,layer) DVE is_eq/mult one-hot builds (~1.0 ms).
  - Self-loop: identity matmul accumulated into the same PSUM group (h'_t
    contributes dinv_i h_i; with the outer dinv_i scale this is dinv_i^2 h_i).
    Bias (zero in this model) would go in as a rank-1 K=1 matmul.
  - Messages/tables/weights in bf16: fp32 matmuls stream at half rate
    (539 ns vs ~220 ns for N=512) and double the gather bytes.
  - ONE dma_gather per (tile, layer) fetches all C*128 edge rows, replacing
    C indirect_dma_start calls (994 ns fixed SWDGE cost each; the 2916
    indirect DMAs were 4.2 ms of serialized GpSimd time in v2).
  - Layer 5/6 tables stay f32 (64-wide bf16 rows would violate dma_gather's
    256B row-size minimum); their messages are DVE-cast to bf16 for the PE.
"""
import sys
import time

sys.path.insert(0, "/opt/trn_rl_repo")
import numpy as np
import ml_dtypes
from contextlib import ExitStack

import concourse.bass as bass
import concourse.mybir as mybir
import concourse.tile as tile
from concourse import library_config
from concourse.bass_utils import run_bass_kernel_spmd
from concourse.masks import make_identity

P = 128
F32 = mybir.dt.float32
BF16 = mybir.dt.bfloat16
I16 = mybir.dt.int16
BF = ml_dtypes.bfloat16

_msw_ctr = [0]


def _split_multiwaits(nc, max_waits=1):
    """This walrus build rejects >1 sync wait per instruction: split extras
    onto preceding same-engine NOPs."""
    for f in nc.m.functions:
        for b in f.blocks:
            out, changed = [], False
            for inst in b.instructions:
                si = getattr(inst, "sync_info", None)
                waits = list(si.on_wait) if si is not None else []
                if len(waits) > max_waits:
                    changed = True
                    for w in waits[:-max_waits]:
                        _msw_ctr[0] += 1
                        nop = mybir.InstNoOp(name=f"msw-{_msw_ctr[0]}", ins=[], outs=[])
                        nop.engine = inst.engine
                        nop.sync_info = mybir.SyncInfo(on_wait=[w], on_update=[])
                        out.append(nop)
                    si.on_wait = waits[-max_waits:]
                out.append(inst)
            if changed:
                b.instructions = out
    return nc


def _pack_graph(src, dst, N):
    """Relabel nodes into degree-balanced 128-node tiles (no self-loops in the
    edge list). Returns raw per-tile edge tables."""
    T = (N + P - 1) // P
    NP = T * P
    indeg = np.bincount(dst, minlength=N)          # no-loop in-degree
    C = max(1, int(np.ceil(len(src) / (T * P))))

    order = np.argsort(-indeg, kind="stable")
    while True:
        cap = C * P
        load = np.zeros(T, np.int64)
        count = np.zeros(T, np.int64)
        assign = np.empty(N, np.int64)
        ok = True
        for v in order:
            d = int(indeg[v])
            best_t, best_rem = -1, -1
            for t in range(T):
                if count[t] < P:
                    rem = cap - load[t]
                    if rem > best_rem:
                        best_rem, best_t = rem, t
            if best_t < 0 or load[best_t] + d > cap:
                ok = False
                break
            assign[v] = best_t
            load[best_t] += d
            count[best_t] += 1
        if ok:
            break
        C += 1

    perm = np.full(NP, -1, np.int64)
    new_of_old = np.empty(N, np.int64)
    cursor = np.zeros(T, np.int64)
    for v in range(N):
        t = assign[v]
        nid = t * P + cursor[t]
        cursor[t] += 1
        perm[nid] = v
        new_of_old[v] = nid

    # symmetric normalization (degree INCLUDES self-loops, per GCN)
    deg = (indeg + 1).astype(np.float32)
    dinv = (1.0 / np.sqrt(deg, dtype=np.float32)).astype(np.float32)

    src_n = new_of_old[src]
    dst_n = new_of_old[dst]
    tile_of_e = dst_n // P
    order_e = np.argsort(tile_of_e, kind="stable")
    src_n, dst_n = src_n[order_e], dst_n[order_e]
    tile_of_e = tile_of_e[order_e]

    gsrc = np.zeros((T, C, P), np.int32)
    slot = np.zeros((T, C, P), np.int32)
    ne = np.zeros(T, np.int64)
    starts = np.searchsorted(tile_of_e, np.arange(T + 1))
    for t in range(T):
        lo, hi = starts[t], starts[t + 1]
        n_e = hi - lo
        assert n_e <= C * P, (t, n_e, C * P)
        ne[t] = n_e
        fs = np.zeros(C * P, np.int32)
        fl = np.zeros(C * P, np.int32)
        fs[:n_e] = src_n[lo:hi]
        fl[:n_e] = (dst_n[lo:hi] - t * P)
        gsrc[t] = fs.reshape(C, P)
        slot[t] = fl.reshape(C, P)

    dinv_new = np.zeros(NP, np.float32)
    valid = perm >= 0
    dinv_new[valid] = dinv[perm[valid]]

    return dict(NP=NP, T=T, C=C, perm=perm, gsrc=gsrc, slot=slot, ne=ne,
                dinv_new=dinv_new)


def _build_nc(NP, T, C, FM, F5, FO, has_bias):
    nc = bass.Bass()
    KM = FM // P
    IW = C * P // 16          # idx columns per tile (wrapped into 16 rows)
    Ident = mybir.ActivationFunctionType.Identity
    Relu = mybir.ActivationFunctionType.Relu

    d = {}
    d["xT1"] = nc.dram_tensor("xT1", [3, NP], BF16, kind="ExternalInput")
    d["hc1"] = nc.dram_tensor("hc1", [1, FM], BF16, kind="ExternalInput")
    d["W1v"] = nc.dram_tensor("W1v", [3, FM], BF16, kind="ExternalInput")
    for i in (2, 3, 4):
        d[f"W{i}"] = nc.dram_tensor(f"W{i}", [FM, FM], BF16, kind="ExternalInput")
    d["W5"] = nc.dram_tensor("W5", [FM, P], BF16, kind="ExternalInput")
    d["W6"] = nc.dram_tensor("W6", [P, FO], BF16, kind="ExternalInput")
    d["oh01"] = nc.dram_tensor("oh01", [P, T * C * P], BF16, kind="ExternalInput")
    d["idx16"] = nc.dram_tensor("idx16", [P, T * IW], I16, kind="ExternalInput")
    d["dinvc"] = nc.dram_tensor("dinvc", [P, T], F32, kind="ExternalInput")
    d["dinv2c"] = nc.dram_tensor("dinv2c", [P, T], F32, kind="ExternalInput")
    if has_bias:
        d["invd"] = nc.dram_tensor("invd", [1, NP], BF16, kind="ExternalInput")
        d["brows"] = nc.dram_tensor("brows", [1, 4 * FM + P + FO], BF16,
                                    kind="ExternalInput")
    out_d = nc.dram_tensor("out", [NP, FO], F32, kind="ExternalOutput")

    h512 = [nc.dram_tensor(f"h{i}", [NP, FM], BF16, kind="Internal")
            for i in (1, 2, 3, 4)]
    h5_d = nc.dram_tensor("h5", [NP, P], BF16, kind="Internal")
    x6_d = nc.dram_tensor("x6", [NP, P], BF16, kind="Internal")

    with tile.TileContext(nc) as tc:
        with ExitStack() as ctx:
            res = ctx.enter_context(tc.tile_pool(name="res", bufs=1))
            nc.gpsimd.load_library(library_config.mlp)
            oh_sb = res.tile([P, T * C * P], BF16)
            nc.sync.dma_start(out=oh_sb[:], in_=d["oh01"][:, :])
            idx_sb = res.tile([P, T * IW], I16)
            nc.sync.dma_start(out=idx_sb[:], in_=d["idx16"][:, :])
            dinvc_sb = res.tile([P, T], F32)
            nc.sync.dma_start(out=dinvc_sb[:], in_=d["dinvc"][:, :])
            dinv2c_sb = res.tile([P, T], F32)
            nc.sync.dma_start(out=dinv2c_sb[:], in_=d["dinv2c"][:, :])
            ident_bf = res.tile([P, P], BF16)
            make_identity(nc, ident_bf[:])
            ident_f = res.tile([P, P], F32)
            make_identity(nc, ident_f[:])
            ones1 = res.tile([1, P], BF16)
            nc.gpsimd.memset(ones1[:], 1.0)
            nreg = nc.gpsimd.to_reg(2 * C * P)
            nreg1 = nc.gpsimd.to_reg(C * P)
            if has_bias:
                invd_sb = res.tile([1, NP], BF16)
                nc.sync.dma_start(out=invd_sb[:], in_=d["invd"][:, :])
                brows_sb = res.tile([1, 4 * FM + F5 + FO], BF16)
                nc.sync.dma_start(out=brows_sb[:], in_=d["brows"][:, :])

            def oh(t, c):
                s = (t * C + c) * P
                return oh_sb[:, s:s + P]

            # ---- layer 1 dense: h'1 = dinv (.) (x W1 + 1 hc^T) ----
            with tc.tile_pool(name="l1", bufs=1) as l1p, \
                 tc.tile_pool(name="l1ps", bufs=2, space="PSUM") as l1ps, \
                 tc.tile_pool(name="l1sb", bufs=3) as l1sb:
                xT1_sb = l1p.tile([3, NP], BF16)
                nc.sync.dma_start(out=xT1_sb[:], in_=d["xT1"][:, :])
                W1v_sb = l1p.tile([3, FM], BF16)
                nc.sync.dma_start(out=W1v_sb[:], in_=d["W1v"][:, :])
                hc1_sb = l1p.tile([1, FM], BF16)
                nc.sync.dma_start(out=hc1_sb[:], in_=d["hc1"][:, :])
                for t in range(T):
                    ph = l1ps.tile([P, FM], F32, tag="ph")
                    nc.tensor.matmul(out=ph[:], lhsT=xT1_sb[:, t * P:(t + 1) * P],
                                     rhs=W1v_sb[:], start=True, stop=False)
                    nc.tensor.matmul(out=ph[:], lhsT=ones1[:], rhs=hc1_sb[:],
                                     start=False, stop=True)
                    hs = l1sb.tile([P, FM], BF16, tag="hs")
                    nc.scalar.activation(out=hs[:], in_=ph[:], func=Ident,
                                         scale=dinvc_sb[:, t:t + 1])
                    nc.sync.dma_start(out=h512[0][t * P:(t + 1) * P, :], in_=hs[:])

            # ---- merged phases: A(layer i) + dense(i+1), i = 1..4 ----
            for i in (1, 2, 3, 4):
                relu = i in (2, 4)
                h_src = h512[i - 1]
                F_out = FM if i < 4 else P
                h_dst = h512[i] if i < 4 else h5_d
                dst_dt = BF16
                W_d = d[f"W{i + 1}"]
                with tc.tile_pool(name=f"ph{i}", bufs=3) as sp, \
                     tc.tile_pool(name=f"ph{i}w", bufs=1) as wp, \
                     tc.tile_pool(name=f"ph{i}ps", bufs=2, space="PSUM") as pp, \
                     tc.tile_pool(name=f"ph{i}pt", bufs=2, space="PSUM") as pt, \
                     tc.tile_pool(name=f"ph{i}pd", bufs=2, space="PSUM") as pd:
                    W_sb = [wp.tile([P, F_out], BF16, tag=f"w{k}", name=f"w{i}_{k}")
                            for k in range(KM)]
                    for k in range(KM):
                        nc.sync.dma_start(out=W_sb[k][:], in_=W_d[k * P:(k + 1) * P, :])
                    msgs_of = {}
                    for t in range(T):
                        if t % 1 == 0:
                            nt = min(1, T - t)
                            msgs = sp.tile([P, nt * C * FM], BF16, tag="msg",
                                           name=f"m{i}_{t}")
                            nc.gpsimd.dma_gather(
                                out_ap=msgs[:].rearrange("p (c f) -> p c f",
                                                         c=nt * C),
                                in_ap=h_src[:, :],
                                idxs_ap=idx_sb[:, t * IW:(t + nt) * IW],
                                num_idxs=nt * C * P,
                                num_idxs_reg=nreg if nt == 2 else nreg1,
                                elem_size=FM,
                            )
                            msgs_of[t] = (msgs, 0)
                            if nt == 2:
                                msgs_of[t + 1] = (msgs, C)
                        msgs, c0 = msgs_of.pop(t)
                        hre = sp.tile([P, FM], BF16, tag="hre", name=f"hre{i}_{t}")
                        nc.sync.dma_start(out=hre[:], in_=h_src[t * P:(t + 1) * P, :])
                        pa = pp.tile([P, FM], F32, tag="pa", name=f"pa{i}_{t}")
                        for c in range(C):
                            nc.tensor.matmul(
                                out=pa[:], lhsT=oh(t, c),
                                rhs=msgs[:, (c0 + c) * FM:(c0 + c + 1) * FM],
                                start=(c == 0), stop=False)
                        nc.tensor.matmul(out=pa[:], lhsT=ident_bf[:], rhs=hre[:],
                                         start=False, stop=not has_bias)
                        if has_bias:
                            nc.tensor.matmul(
                                out=pa[:],
                                lhsT=invd_sb[:, t * P:(t + 1) * P],
                                rhs=brows_sb[:, (i - 1) * FM:i * FM],
                                start=False, stop=True)
                        node = sp.tile([P, FM], BF16, tag="node", name=f"nd{i}_{t}")
                        nc.scalar.activation(out=node[:], in_=pa[:],
                                             func=Relu if relu else Ident,
                                             scale=dinvc_sb[:, t:t + 1])
                        ptr = pt.tile([P, FM], BF16, tag="ptr", name=f"pt{i}_{t}")
                        stage = sp.tile([P, FM], BF16, tag="stage", name=f"st{i}_{t}")
                        for fo in range(KM):
                            nc.tensor.matmul(
                                out=ptr[:, fo * P:(fo + 1) * P],
                                lhsT=node[:, fo * P:(fo + 1) * P],
                                rhs=ident_bf[:], is_transpose=True,
                                start=True, stop=True)
                            nc.vector.tensor_copy(
                                out=stage[:, fo * P:(fo + 1) * P],
                                in_=ptr[:, fo * P:(fo + 1) * P])
                        pdt = pd.tile([P, F_out], F32, tag="pd", name=f"pd{i}_{t}")
                        for k in range(KM):
                            nc.tensor.matmul(out=pdt[:], lhsT=stage[:, k * P:(k + 1) * P],
                                             rhs=W_sb[k][:], start=(k == 0),
                                             stop=(k == KM - 1))
                        hs = sp.tile([P, F_out], dst_dt, tag="hs", name=f"hs{i}_{t}")
                        nc.vector.tensor_scalar_mul(
                            out=hs[:], in0=pdt[:],
                            scalar1=dinvc_sb[:, t:t + 1])
                        nc.sync.dma_start(out=h_dst[t * P:(t + 1) * P, :], in_=hs[:])

            # ---- phase 5: A(layer 5) -> x6' = dinv^2 (.) pa5 (128-wide pad) ----
            with tc.tile_pool(name="s5", bufs=3) as sp5, \
                 tc.tile_pool(name="s5ps", bufs=2, space="PSUM") as pp5:
                m5_of = {}
                for t in range(T):
                    if t % 1 == 0:
                        nt = min(1, T - t)
                        m5 = sp5.tile([P, nt * C * P], BF16, tag="m5",
                                      name=f"m5_{t}")
                        nc.gpsimd.dma_gather(
                            out_ap=m5[:].rearrange("p (c f) -> p c f", c=nt * C),
                            in_ap=h5_d[:, :],
                            idxs_ap=idx_sb[:, t * IW:(t + nt) * IW],
                            num_idxs=nt * C * P,
                            num_idxs_reg=nreg if nt == 2 else nreg1,
                            elem_size=P,
                        )
                        m5_of[t] = (m5, 0)
                        if nt == 2:
                            m5_of[t + 1] = (m5, C)
                    m5, c0 = m5_of.pop(t)
                    h5t = sp5.tile([P, P], BF16, tag="h5t", name=f"h5t_{t}")
                    nc.sync.dma_start(out=h5t[:], in_=h5_d[t * P:(t + 1) * P, :])
                    pa5 = pp5.tile([P, P], F32, tag="pa5", name=f"pa5_{t}")
                    for c in range(C):
                        nc.tensor.matmul(out=pa5[:], lhsT=oh(t, c),
                                         rhs=m5[:, (c0 + c) * P:(c0 + c + 1) * P],
                                         start=(c == 0), stop=False)
                    nc.tensor.matmul(out=pa5[:], lhsT=ident_bf[:], rhs=h5t[:],
                                     start=False, stop=not has_bias)
                    if has_bias:
                        nc.tensor.matmul(
                            out=pa5[:],
                            lhsT=invd_sb[:, t * P:(t + 1) * P],
                            rhs=brows_sb[:, 4 * FM:4 * FM + P],
                            start=False, stop=True)
                    x6s = sp5.tile([P, P], BF16, tag="x6s", name=f"x6s_{t}")
                    nc.scalar.activation(out=x6s[:], in_=pa5[:], func=Ident,
                                         scale=dinv2c_sb[:, t:t + 1])
                    nc.sync.dma_start(out=x6_d[t * P:(t + 1) * P, :], in_=x6s[:])

            # ---- phase 6: A(layer 6) feature-major + dense W6 + out ----
            with tc.tile_pool(name="s6", bufs=3) as sp6, \
                 tc.tile_pool(name="s6w", bufs=1) as wp6, \
                 tc.tile_pool(name="s6ps", bufs=2, space="PSUM") as pp6, \
                 tc.tile_pool(name="s6pd", bufs=2, space="PSUM") as pd6:
                W6_sb = wp6.tile([P, FO], BF16)
                nc.sync.dma_start(out=W6_sb[:], in_=d["W6"][:, :])
                m6_of = {}
                for t in range(T):
                    if t % 1 == 0:
                        nt = min(1, T - t)
                        m6 = sp6.tile([P, nt * C * P], BF16, tag="m6",
                                      name=f"m6_{t}")
                        nc.gpsimd.dma_gather(
                            out_ap=m6[:].rearrange("p (c f) -> p c f", c=nt * C),
                            in_ap=x6_d[:, :],
                            idxs_ap=idx_sb[:, t * IW:(t + nt) * IW],
                            num_idxs=nt * C * P,
                            num_idxs_reg=nreg if nt == 2 else nreg1,
                            elem_size=P,
                        )
                        m6_of[t] = (m6, 0)
                        if nt == 2:
                            m6_of[t + 1] = (m6, C)
                    m6, c0 = m6_of.pop(t)
                    x6t = sp6.tile([P, P], BF16, tag="x6t", name=f"x6t_{t}")
                    nc.sync.dma_start(out=x6t[:], in_=x6_d[t * P:(t + 1) * P, :])
                    pg = pp6.tile([P, P], F32, tag="pg", name=f"pg_{t}")
                    nc.tensor.matmul(out=pg[:], lhsT=x6t[:], rhs=ident_bf[:],
                                     start=True, stop=False)
                    for c in range(C):
                        nc.tensor.matmul(
                            out=pg[:], lhsT=m6[:, (c0 + c) * P:(c0 + c + 1) * P],
                            rhs=oh(t, c),
                            start=False, stop=(c == C - 1))
                    gst = sp6.tile([P, P], BF16, tag="gst", name=f"g_{t}")
                    nc.vector.tensor_copy(out=gst[:], in_=pg[:])
                    pf = pd6.tile([P, FO], F32, tag="pf", name=f"pf_{t}")
                    nc.tensor.matmul(out=pf[:], lhsT=gst[:], rhs=W6_sb[:],
                                     start=True, stop=not has_bias)
                    if has_bias:
                        nc.tensor.matmul(
                            out=pf[:],
                            lhsT=invd_sb[:, t * P:(t + 1) * P],
                            rhs=brows_sb[:, 4 * FM + P:],
                            start=False, stop=True)
                    osb = sp6.tile([P, FO], F32, tag="os", name=f"o_{t}")
                    nc.scalar.activation(out=osb[:], in_=pf[:], func=Ident,
                                         scale=dinvc_sb[:, t:t + 1])
                    nc.sync.dma_start(out=out_d[t * P:(t + 1) * P, :], in_=osb[:])

    _postprocess(nc)
    return nc


def _postprocess(nc):
    """HW lowering: split multi-waits (this walrus rejects >1/instruction) and
    encode ISA-subclass instructions (library reload, dma_gather) to bytes.
    simcheck.py stubs this out — CoreSim interprets the typed forms."""
    _split_multiwaits(nc)
    mybir.codegen_inst_isa_subclasses(nc)
    return nc


def _prepare(batch_vertices, img_features, edge_indices,
             W1, b1, W2, b2, W3, b3, W4, b4, W5, b5, W6, b6):
    B, N, _ = batch_vertices.shape
    FM = W1.shape[1]
    F5 = W5.shape[1]
    FO = W6.shape[1]

    ei = np.asarray(edge_indices).astype(np.int64)
    g = _pack_graph(ei[0], ei[1], N)
    NP, T, C, perm = g["NP"], g["T"], g["C"], g["perm"]
    gsrc, slot, ne, dinv_new = g["gsrc"], g["slot"], g["ne"], g["dinv_new"]

    # host one-hot (0/1, dummy edge slots masked)
    j = np.arange(P, dtype=np.int32)
    oh = (slot[..., None] == j).astype(np.float32)          # [T, C, P, 128]
    eslot = (np.arange(C * P).reshape(C, P))
    valid_e = (eslot[None, :, :] < ne[:, None, None])       # [T, C, P]
    oh *= valid_e[..., None]
    oh_dev = np.ascontiguousarray(
        oh.transpose(2, 0, 1, 3).reshape(P, T * C * P)).astype(BF)

    # wrapped int16 gather indices: idx i of tile t at [i%16, t*IW + i//16]
    IW = C * P // 16
    flat = gsrc.reshape(T, C * P).astype(np.int16)          # c-major per tile
    wrapped = flat.reshape(T, IW, 16).transpose(0, 2, 1)    # [T, 16, IW]
    wrapped = np.tile(wrapped, (1, 8, 1))                   # [T, 128, IW]
    idx16 = np.ascontiguousarray(wrapped.transpose(1, 0, 2).reshape(P, T * IW))

    dinvc = np.ascontiguousarray(dinv_new.reshape(T, P).T)
    dinv2c = np.ascontiguousarray((dinv_new ** 2).reshape(T, P).T)

    biases = [np.asarray(b, np.float32) for b in (b2, b3, b4, b5, b6)]
    has_bias = any(np.abs(b).max() > 0 for b in biases)

    hc = (img_features.astype(np.float32) @ W1[3:].astype(np.float32)
          + b1.astype(np.float32)[None, :])

    valid = perm >= 0
    vperm = np.zeros((B, NP, 3), np.float32)
    vperm[:, valid, :] = batch_vertices[:, perm[valid], :]

    common = {
        "W1v": np.ascontiguousarray(W1[:3]).astype(BF),
        "W2": np.ascontiguousarray(W2).astype(BF),
        "W3": np.ascontiguousarray(W3).astype(BF),
        "W4": np.ascontiguousarray(W4).astype(BF),
        "W5": np.ascontiguousarray(
            np.pad(np.asarray(W5, np.float32), ((0, 0), (0, 128 - W5.shape[1])))
        ).astype(BF),
        "W6": np.ascontiguousarray(
            np.pad(np.asarray(W6, np.float32), ((0, 128 - W6.shape[0]), (0, 0)))
        ).astype(BF),
        "oh01": oh_dev, "idx16": idx16,
        "dinvc": dinvc, "dinv2c": dinv2c,
    }
    if has_bias:
        invd = np.zeros(NP, np.float32)
        invd[valid] = 1.0 / dinv_new[valid]
        common["invd"] = invd[None, :].astype(BF)
        bpad = biases[:3] + [np.pad(biases[3], (0, 128 - biases[3].size)),
                             biases[4]]
        common["brows"] = np.concatenate(bpad)[None, :].astype(BF)
    in_maps = []
    for b in range(B):
        m = dict(common)
        m["xT1"] = np.ascontiguousarray(vperm[b].T).astype(BF)
        m["hc1"] = hc[b][None, :].astype(BF)
        in_maps.append(m)
    meta = dict(NP=NP, T=T, C=C, perm=perm, valid=valid, B=B, N=N,
                FM=FM, F5=F5, FO=FO, has_bias=has_bias)
    return in_maps, meta


_BUILD_CACHE = {}


def run(inputs, trace=False):
    in_maps, meta = _prepare(**inputs)
    key = (meta["NP"], meta["C"], meta["FM"], meta["F5"], meta["FO"],
           meta["has_bias"])
    if key not in _BUILD_CACHE:
        t0 = time.time()
        _BUILD_CACHE[key] = _build_nc(meta["NP"], meta["T"], meta["C"],
                                      meta["FM"], meta["F5"], meta["FO"],
                                      meta["has_bias"])
        print(f"[kernel] built bass program in {time.time()-t0:.1f}s", file=sys.stderr)
    nc = _BUILD_CACHE[key]
    B = meta["B"]
    res = run_bass_kernel_spmd(nc, in_maps, core_ids=list(range(B)), trace=trace)
    perm, valid, N = meta["perm"], meta["valid"], meta["N"]
    out = np.empty((B, N, meta["FO"]), np.float32)
    for b in range(B):
        dev = res.results[b]["out"]
        out[b, perm[valid], :] = dev[valid, :]
    return out, res


def kernel(**inputs) -> np.ndarray:
    out, _ = run(inputs)
    return out


# revision 14
# speedup vs baseline: 1.2078x; 1.0830x over previous
"""GCN (6-layer GCNConv) Trainium2 Bass kernel — v3.

Data-parallel over batch (1 mesh per NeuronCore). Per layer
out = A_hat @ (x @ W) + b with A_hat = D^-1/2 (A+I) D^-1/2 shared across batch
and layers.

v3 changes vs v2 (trace-driven):
  - The symmetric norm factorizes: norm_e = dinv[src]*dinv[dst]. Tables store
    h' = dinv (.) (x W); the one-hot segment-sum matrices become 0/1 and are
    precomputed on the HOST and uploaded once (bf16, exact). The dst-side dinv
    is fused into the PSUM evacuation via scalar.activation(scale=AP).
    Kills the per-(tile,layer) DVE is_eq/mult one-hot builds (~1.0 ms).
  - Self-loop: identity matmul accumulated into the same PSUM group (h'_t
    contributes dinv_i h_i; with the outer dinv_i scale this is dinv_i^2 h_i).
    Bias (zero in this model) would go in as a rank-1 K=1 matmul.
  - Messages/tables/weights in bf16: fp32 matmuls stream at half rate
    (539 ns vs ~220 ns for N=512) and double the gather bytes.
  - ONE dma_gather per (tile, layer) fetches all C*128 edge rows, replacing
    C indirect_dma_start calls (994 ns fixed SWDGE cost each; the 2916
    indirect DMAs were 4.2 ms of serialized GpSimd time in v2).
  - Layer 5/6 tables stay f32 (64-wide bf16 rows would violate dma_gather's
    256B row-size minimum); their messages are DVE-cast to bf16 for the PE.
"""
import sys
import time

sys.path.insert(0, "/opt/trn_rl_repo")
import numpy as np
import ml_dtypes
from contextlib import ExitStack

import concourse.bass as bass
import concourse.mybir as mybir
import concourse.tile as tile
from concourse import library_config
from concourse.bass_utils import run_bass_kernel_spmd
from concourse.masks import make_identity

P = 128
F32 = mybir.dt.float32
BF16 = mybir.dt.bfloat16
I16 = mybir.dt.int16
BF = ml_dtypes.bfloat16

_msw_ctr = [0]


def _split_multiwaits(nc, max_waits=1):
    """This walrus build rejects >1 sync wait per instruction: split extras
    onto preceding same-engine NOPs."""
    for f in nc.m.functions:
        for b in f.blocks:
            out, changed = [], False
            for inst in b.instructions:
                si = getattr(inst, "sync_info", None)
                waits = list(si.on_wait) if si is not None else []
                if len(waits) > max_waits:
                    changed = True
                    for w in waits[:-max_waits]:
                        _msw_ctr[0] += 1
                        nop = mybir.InstNoOp(name=f"msw-{_msw_ctr[0]}", ins=[], outs=[])
                        nop.engine = inst.engine
                        nop.sync_info = mybir.SyncInfo(on_wait=[w], on_update=[])
                        out.append(nop)
                    si.on_wait = waits[-max_waits:]
                out.append(inst)
            if changed:
                b.instructions = out
    return nc


def _pack_graph(src, dst, N):
    """Relabel nodes into degree-balanced 128-node tiles (no self-loops in the
    edge list). Returns raw per-tile edge tables."""
    T = (N + P - 1) // P
    NP = T * P
    indeg = np.bincount(dst, minlength=N)          # no-loop in-degree
    C = max(1, int(np.ceil(len(src) / (T * P))))

    order = np.argsort(-indeg, kind="stable")
    while True:
        cap = C * P
        load = np.zeros(T, np.int64)
        count = np.zeros(T, np.int64)
        assign = np.empty(N, np.int64)
        ok = True
        for v in order:
            d = int(indeg[v])
            best_t, best_rem = -1, -1
            for t in range(T):
                if count[t] < P:
                    rem = cap - load[t]
                    if rem > best_rem:
                        best_rem, best_t = rem, t
            if best_t < 0 or load[best_t] + d > cap:
                ok = False
                break
            assign[v] = best_t
            load[best_t] += d
            count[best_t] += 1
        if ok:
            break
        C += 1

    perm = np.full(NP, -1, np.int64)
    new_of_old = np.empty(N, np.int64)
    cursor = np.zeros(T, np.int64)
    for v in range(N):
        t = assign[v]
        nid = t * P + cursor[t]
        cursor[t] += 1
        perm[nid] = v
        new_of_old[v] = nid

    # symmetric normalization (degree INCLUDES self-loops, per GCN)
    deg = (indeg + 1).astype(np.float32)
    dinv = (1.0 / np.sqrt(deg, dtype=np.float32)).astype(np.float32)

    src_n = new_of_old[src]
    dst_n = new_of_old[dst]
    tile_of_e = dst_n // P
    order_e = np.argsort(tile_of_e, kind="stable")
    src_n, dst_n = src_n[order_e], dst_n[order_e]
    tile_of_e = tile_of_e[order_e]

    gsrc = np.zeros((T, C, P), np.int32)
    slot = np.zeros((T, C, P), np.int32)
    ne = np.zeros(T, np.int64)
    starts = np.searchsorted(tile_of_e, np.arange(T + 1))
    for t in range(T):
        lo, hi = starts[t], starts[t + 1]
        n_e = hi - lo
        assert n_e <= C * P, (t, n_e, C * P)
        ne[t] = n_e
        fs = np.zeros(C * P, np.int32)
        fl = np.zeros(C * P, np.int32)
        fs[:n_e] = src_n[lo:hi]
        fl[:n_e] = (dst_n[lo:hi] - t * P)
        gsrc[t] = fs.reshape(C, P)
        slot[t] = fl.reshape(C, P)

    dinv_new = np.zeros(NP, np.float32)
    valid = perm >= 0
    dinv_new[valid] = dinv[perm[valid]]

    return dict(NP=NP, T=T, C=C, perm=perm, gsrc=gsrc, slot=slot, ne=ne,
                dinv_new=dinv_new)


def _build_nc(NP, T, C, FM, F5, FO, has_bias):
    nc = bass.Bass()
    KM = FM // P
    IW = C * P // 16          # idx columns per tile (wrapped into 16 rows)
    ICW = P // 16             # idx columns per 128-idx chunk
    Ident = mybir.ActivationFunctionType.Identity
    Relu = mybir.ActivationFunctionType.Relu

    d = {}
    d["xT1"] = nc.dram_tensor("xT1", [3, NP], BF16, kind="ExternalInput")
    d["hc1"] = nc.dram_tensor("hc1", [1, FM], BF16, kind="ExternalInput")
    d["W1v"] = nc.dram_tensor("W1v", [3, FM], BF16, kind="ExternalInput")
    for i in (2, 3, 4):
        d[f"W{i}"] = nc.dram_tensor(f"W{i}", [FM, FM], BF16, kind="ExternalInput")
    d["W5"] = nc.dram_tensor("W5", [FM, P], BF16, kind="ExternalInput")
    d["W6"] = nc.dram_tensor("W6", [P, FO], BF16, kind="ExternalInput")
    d["oh01"] = nc.dram_tensor("oh01", [P, T * C * P], BF16, kind="ExternalInput")
    d["idx16"] = nc.dram_tensor("idx16", [P, T * IW], I16, kind="ExternalInput")
    d["dinvc"] = nc.dram_tensor("dinvc", [P, T], F32, kind="ExternalInput")
    d["dinv2c"] = nc.dram_tensor("dinv2c", [P, T], F32, kind="ExternalInput")
    if has_bias:
        d["invd"] = nc.dram_tensor("invd", [1, NP], BF16, kind="ExternalInput")
        d["brows"] = nc.dram_tensor("brows", [1, 4 * FM + P + FO], BF16,
                                    kind="ExternalInput")
    out_d = nc.dram_tensor("out", [NP, FO], F32, kind="ExternalOutput")

    h512 = [nc.dram_tensor(f"h{i}", [NP, FM], BF16, kind="Internal")
            for i in (1, 2, 3, 4)]
    h5_d = nc.dram_tensor("h5", [NP, P], BF16, kind="Internal")
    x6_d = nc.dram_tensor("x6", [NP, P], BF16, kind="Internal")

    with tile.TileContext(nc) as tc:
        with ExitStack() as ctx:
            res = ctx.enter_context(tc.tile_pool(name="res", bufs=1))
            nc.gpsimd.load_library(library_config.mlp)
            oh_sb = res.tile([P, T * C * P], BF16)
            nc.sync.dma_start(out=oh_sb[:], in_=d["oh01"][:, :])
            idx_sb = res.tile([P, T * IW], I16)
            nc.sync.dma_start(out=idx_sb[:], in_=d["idx16"][:, :])
            dinvc_sb = res.tile([P, T], F32)
            nc.sync.dma_start(out=dinvc_sb[:], in_=d["dinvc"][:, :])
            dinv2c_sb = res.tile([P, T], F32)
            nc.sync.dma_start(out=dinv2c_sb[:], in_=d["dinv2c"][:, :])
            ident_bf = res.tile([P, P], BF16)
            make_identity(nc, ident_bf[:])
            ident_f = res.tile([P, P], F32)
            make_identity(nc, ident_f[:])
            ones1 = res.tile([1, P], BF16)
            nc.gpsimd.memset(ones1[:], 1.0)
            nreg = nc.gpsimd.to_reg(8 * P)
            nreg1 = nc.gpsimd.to_reg((T * C % 8) * P)
            if has_bias:
                invd_sb = res.tile([1, NP], BF16)
                nc.sync.dma_start(out=invd_sb[:], in_=d["invd"][:, :])
                brows_sb = res.tile([1, 4 * FM + F5 + FO], BF16)
                nc.sync.dma_start(out=brows_sb[:], in_=d["brows"][:, :])

            def oh(t, c):
                s = (t * C + c) * P
                return oh_sb[:, s:s + P]

            # ---- layer 1 dense: h'1 = dinv (.) (x W1 + 1 hc^T) ----
            with tc.tile_pool(name="l1", bufs=1) as l1p, \
                 tc.tile_pool(name="l1ps", bufs=2, space="PSUM") as l1ps, \
                 tc.tile_pool(name="l1sb", bufs=3) as l1sb:
                xT1_sb = l1p.tile([3, NP], BF16)
                nc.sync.dma_start(out=xT1_sb[:], in_=d["xT1"][:, :])
                W1v_sb = l1p.tile([3, FM], BF16)
                nc.sync.dma_start(out=W1v_sb[:], in_=d["W1v"][:, :])
                hc1_sb = l1p.tile([1, FM], BF16)
                nc.sync.dma_start(out=hc1_sb[:], in_=d["hc1"][:, :])
                for t in range(T):
                    ph = l1ps.tile([P, FM], F32, tag="ph")
                    nc.tensor.matmul(out=ph[:], lhsT=xT1_sb[:, t * P:(t + 1) * P],
                                     rhs=W1v_sb[:], start=True, stop=False)
                    nc.tensor.matmul(out=ph[:], lhsT=ones1[:], rhs=hc1_sb[:],
                                     start=False, stop=True)
                    hs = l1sb.tile([P, FM], BF16, tag="hs")
                    nc.scalar.activation(out=hs[:], in_=ph[:], func=Ident,
                                         scale=dinvc_sb[:, t:t + 1])
                    nc.sync.dma_start(out=h512[0][t * P:(t + 1) * P, :], in_=hs[:])

            # ---- merged phases: A(layer i) + dense(i+1), i = 1..4 ----
            for i in (1, 2, 3, 4):
                relu = i in (2, 4)
                h_src = h512[i - 1]
                F_out = FM if i < 4 else P
                h_dst = h512[i] if i < 4 else h5_d
                dst_dt = BF16
                W_d = d[f"W{i + 1}"]
                with tc.tile_pool(name=f"ph{i}", bufs=3) as sp, \
                     tc.tile_pool(name=f"ph{i}w", bufs=1) as wp, \
                     tc.tile_pool(name=f"ph{i}ps", bufs=2, space="PSUM") as pp, \
                     tc.tile_pool(name=f"ph{i}pt", bufs=2, space="PSUM") as pt, \
                     tc.tile_pool(name=f"ph{i}pd", bufs=2, space="PSUM") as pd:
                    W_sb = [wp.tile([P, F_out], BF16, tag=f"w{k}", name=f"w{i}_{k}")
                            for k in range(KM)]
                    for k in range(KM):
                        nc.sync.dma_start(out=W_sb[k][:], in_=W_d[k * P:(k + 1) * P, :])
                    NCH = T * C
                    gbuf = {}
                    next_g = 0

                    def gather_upto(j, src_t, F_e):
                        nonlocal next_g
                        while next_g * 8 <= j:
                            g = next_g
                            nch = min(8, NCH - g * 8)
                            mb = sp.tile([P, 8 * F_e], BF16, tag="msg",
                                         name=f"mg{i}_{g}")
                            nc.gpsimd.dma_gather(
                                out_ap=mb[:, :nch * F_e].rearrange(
                                    "p (c f) -> p c f", c=nch),
                                in_ap=src_t[:, :],
                                idxs_ap=idx_sb[:, g * 8 * ICW:(g * 8 + nch) * ICW],
                                num_idxs=nch * P,
                                num_idxs_reg=nreg if nch == 8 else nreg1,
                                elem_size=F_e,
                            )
                            gbuf[g] = mb
                            gbuf.pop(g - 4, None)
                            next_g += 1

                    for t in range(T):
                        gather_upto(t * C + C - 1, h_src, FM)
                        hre = sp.tile([P, FM], BF16, tag="hre", name=f"hre{i}_{t}")
                        nc.sync.dma_start(out=hre[:], in_=h_src[t * P:(t + 1) * P, :])
                        pa = pp.tile([P, FM], F32, tag="pa", name=f"pa{i}_{t}")
                        for c in range(C):
                            j = t * C + c
                            mb = gbuf[j // 8]
                            nc.tensor.matmul(
                                out=pa[:], lhsT=oh(t, c),
                                rhs=mb[:, (j % 8) * FM:(j % 8 + 1) * FM],
                                start=(c == 0), stop=False)
                        nc.tensor.matmul(out=pa[:], lhsT=ident_bf[:], rhs=hre[:],
                                         start=False, stop=not has_bias)
                        if has_bias:
                            nc.tensor.matmul(
                                out=pa[:],
                                lhsT=invd_sb[:, t * P:(t + 1) * P],
                                rhs=brows_sb[:, (i - 1) * FM:i * FM],
                                start=False, stop=True)
                        node = sp.tile([P, FM], BF16, tag="node", name=f"nd{i}_{t}")
                        nc.scalar.activation(out=node[:], in_=pa[:],
                                             func=Relu if relu else Ident,
                                             scale=dinvc_sb[:, t:t + 1])
                        ptr = pt.tile([P, FM], BF16, tag="ptr", name=f"pt{i}_{t}")
                        stage = sp.tile([P, FM], BF16, tag="stage", name=f"st{i}_{t}")
                        for fo in range(KM):
                            nc.tensor.matmul(
                                out=ptr[:, fo * P:(fo + 1) * P],
                                lhsT=node[:, fo * P:(fo + 1) * P],
                                rhs=ident_bf[:], is_transpose=True,
                                start=True, stop=True)
                            nc.vector.tensor_copy(
                                out=stage[:, fo * P:(fo + 1) * P],
                                in_=ptr[:, fo * P:(fo + 1) * P])
                        pdt = pd.tile([P, F_out], F32, tag="pd", name=f"pd{i}_{t}")
                        for k in range(KM):
                            nc.tensor.matmul(out=pdt[:], lhsT=stage[:, k * P:(k + 1) * P],
                                             rhs=W_sb[k][:], start=(k == 0),
                                             stop=(k == KM - 1))
                        hs = sp.tile([P, F_out], dst_dt, tag="hs", name=f"hs{i}_{t}")
                        nc.vector.tensor_scalar_mul(
                            out=hs[:], in0=pdt[:],
                            scalar1=dinvc_sb[:, t:t + 1])
                        nc.sync.dma_start(out=h_dst[t * P:(t + 1) * P, :], in_=hs[:])

            # ---- phase 5: A(layer 5) -> x6' = dinv^2 (.) pa5 (128-wide pad) ----
            with tc.tile_pool(name="s5", bufs=3) as sp5, \
                 tc.tile_pool(name="s5ps", bufs=2, space="PSUM") as pp5:
                NCH = T * C
                gbuf5 = {}
                next_g5 = 0

                def gather5_upto(j):
                    nonlocal next_g5
                    while next_g5 * 8 <= j:
                        g = next_g5
                        nch = min(8, NCH - g * 8)
                        mb = sp5.tile([P, 8 * P], BF16, tag="m5", name=f"m5g_{g}")
                        nc.gpsimd.dma_gather(
                            out_ap=mb[:, :nch * P].rearrange(
                                "p (c f) -> p c f", c=nch),
                            in_ap=h5_d[:, :],
                            idxs_ap=idx_sb[:, g * 8 * ICW:(g * 8 + nch) * ICW],
                            num_idxs=nch * P,
                            num_idxs_reg=nreg if nch == 8 else nreg1,
                            elem_size=P,
                        )
                        gbuf5[g] = mb
                        gbuf5.pop(g - 4, None)
                        next_g5 += 1

                for t in range(T):
                    gather5_upto(t * C + C - 1)
                    h5t = sp5.tile([P, P], BF16, tag="h5t", name=f"h5t_{t}")
                    nc.sync.dma_start(out=h5t[:], in_=h5_d[t * P:(t + 1) * P, :])
                    pa5 = pp5.tile([P, P], F32, tag="pa5", name=f"pa5_{t}")
                    for c in range(C):
                        j = t * C + c
                        mb = gbuf5[j // 8]
                        nc.tensor.matmul(out=pa5[:], lhsT=oh(t, c),
                                         rhs=mb[:, (j % 8) * P:(j % 8 + 1) * P],
                                         start=(c == 0), stop=False)
                    nc.tensor.matmul(out=pa5[:], lhsT=ident_bf[:], rhs=h5t[:],
                                     start=False, stop=not has_bias)
                    if has_bias:
                        nc.tensor.matmul(
                            out=pa5[:],
                            lhsT=invd_sb[:, t * P:(t + 1) * P],
                            rhs=brows_sb[:, 4 * FM:4 * FM + P],
                            start=False, stop=True)
                    x6s = sp5.tile([P, P], BF16, tag="x6s", name=f"x6s_{t}")
                    nc.scalar.activation(out=x6s[:], in_=pa5[:], func=Ident,
                                         scale=dinv2c_sb[:, t:t + 1])
                    nc.sync.dma_start(out=x6_d[t * P:(t + 1) * P, :], in_=x6s[:])

            # ---- phase 6: A(layer 6) feature-major + dense W6 + out ----
            with tc.tile_pool(name="s6", bufs=3) as sp6, \
                 tc.tile_pool(name="s6w", bufs=1) as wp6, \
                 tc.tile_pool(name="s6ps", bufs=2, space="PSUM") as pp6, \
                 tc.tile_pool(name="s6pd", bufs=2, space="PSUM") as pd6:
                W6_sb = wp6.tile([P, FO], BF16)
                nc.sync.dma_start(out=W6_sb[:], in_=d["W6"][:, :])
                NCH = T * C
                gbuf6 = {}
                next_g6 = 0

                def gather6_upto(j):
                    nonlocal next_g6
                    while next_g6 * 8 <= j:
                        g = next_g6
                        nch = min(8, NCH - g * 8)
                        mb = sp6.tile([P, 8 * P], BF16, tag="m6", name=f"m6g_{g}")
                        nc.gpsimd.dma_gather(
                            out_ap=mb[:, :nch * P].rearrange(
                                "p (c f) -> p c f", c=nch),
                            in_ap=x6_d[:, :],
                            idxs_ap=idx_sb[:, g * 8 * ICW:(g * 8 + nch) * ICW],
                            num_idxs=nch * P,
                            num_idxs_reg=nreg if nch == 8 else nreg1,
                            elem_size=P,
                        )
                        gbuf6[g] = mb
                        gbuf6.pop(g - 4, None)
                        next_g6 += 1

                for t in range(T):
                    gather6_upto(t * C + C - 1)
                    x6t = sp6.tile([P, P], BF16, tag="x6t", name=f"x6t_{t}")
                    nc.sync.dma_start(out=x6t[:], in_=x6_d[t * P:(t + 1) * P, :])
                    pg = pp6.tile([P, P], F32, tag="pg", name=f"pg_{t}")
                    nc.tensor.matmul(out=pg[:], lhsT=x6t[:], rhs=ident_bf[:],
                                     start=True, stop=False)
                    for c in range(C):
                        j = t * C + c
                        mb = gbuf6[j // 8]
                        nc.tensor.matmul(
                            out=pg[:], lhsT=mb[:, (j % 8) * P:(j % 8 + 1) * P],
                            rhs=oh(t, c),
                            start=False, stop=(c == C - 1))
                    gst = sp6.tile([P, P], BF16, tag="gst", name=f"g_{t}")
                    nc.vector.tensor_copy(out=gst[:], in_=pg[:])
                    pf = pd6.tile([P, FO], F32, tag="pf", name=f"pf_{t}")
                    nc.tensor.matmul(out=pf[:], lhsT=gst[:], rhs=W6_sb[:],
                                     start=True, stop=not has_bias)
                    if has_bias:
                        nc.tensor.matmul(
                            out=pf[:],
                            lhsT=invd_sb[:, t * P:(t + 1) * P],
                            rhs=brows_sb[:, 4 * FM + P:],
                            start=False, stop=True)
                    osb = sp6.tile([P, FO], F32, tag="os", name=f"o_{t}")
                    nc.scalar.activation(out=osb[:], in_=pf[:], func=Ident,
                                         scale=dinvc_sb[:, t:t + 1])
                    nc.sync.dma_start(out=out_d[t * P:(t + 1) * P, :], in_=osb[:])

    _postprocess(nc)
    return nc


def _postprocess(nc):
    """HW lowering: split multi-waits (this walrus rejects >1/instruction) and
    encode ISA-subclass instructions (library reload, dma_gather) to bytes.
    simcheck.py stubs this out — CoreSim interprets the typed forms."""
    _split_multiwaits(nc)
    mybir.codegen_inst_isa_subclasses(nc)
    return nc


def _prepare(batch_vertices, img_features, edge_indices,
             W1, b1, W2, b2, W3, b3, W4, b4, W5, b5, W6, b6):
    B, N, _ = batch_vertices.shape
    FM = W1.shape[1]
    F5 = W5.shape[1]
    FO = W6.shape[1]

    ei = np.asarray(edge_indices).astype(np.int64)
    g = _pack_graph(ei[0], ei[1], N)
    NP, T, C, perm = g["NP"], g["T"], g["C"], g["perm"]
    gsrc, slot, ne, dinv_new = g["gsrc"], g["slot"], g["ne"], g["dinv_new"]

    # host one-hot (0/1, dummy edge slots masked)
    j = np.arange(P, dtype=np.int32)
    oh = (slot[..., None] == j).astype(np.float32)          # [T, C, P, 128]
    eslot = (np.arange(C * P).reshape(C, P))
    valid_e = (eslot[None, :, :] < ne[:, None, None])       # [T, C, P]
    oh *= valid_e[..., None]
    oh_dev = np.ascontiguousarray(
        oh.transpose(2, 0, 1, 3).reshape(P, T * C * P)).astype(BF)

    # wrapped int16 gather indices: idx i of tile t at [i%16, t*IW + i//16]
    IW = C * P // 16
    flat = gsrc.reshape(T, C * P).astype(np.int16)          # c-major per tile
    wrapped = flat.reshape(T, IW, 16).transpose(0, 2, 1)    # [T, 16, IW]
    wrapped = np.tile(wrapped, (1, 8, 1))                   # [T, 128, IW]
    idx16 = np.ascontiguousarray(wrapped.transpose(1, 0, 2).reshape(P, T * IW))

    dinvc = np.ascontiguousarray(dinv_new.reshape(T, P).T)
    dinv2c = np.ascontiguousarray((dinv_new ** 2).reshape(T, P).T)

    biases = [np.asarray(b, np.float32) for b in (b2, b3, b4, b5, b6)]
    has_bias = any(np.abs(b).max() > 0 for b in biases)

    hc = (img_features.astype(np.float32) @ W1[3:].astype(np.float32)
          + b1.astype(np.float32)[None, :])

    valid = perm >= 0
    vperm = np.zeros((B, NP, 3), np.float32)
    vperm[:, valid, :] = batch_vertices[:, perm[valid], :]

    common = {
        "W1v": np.ascontiguousarray(W1[:3]).astype(BF),
        "W2": np.ascontiguousarray(W2).astype(BF),
        "W3": np.ascontiguousarray(W3).astype(BF),
        "W4": np.ascontiguousarray(W4).astype(BF),
        "W5": np.ascontiguousarray(
            np.pad(np.asarray(W5, np.float32), ((0, 0), (0, 128 - W5.shape[1])))
        ).astype(BF),
        "W6": np.ascontiguousarray(
            np.pad(np.asarray(W6, np.float32), ((0, 128 - W6.shape[0]), (0, 0)))
        ).astype(BF),
        "oh01": oh_dev, "idx16": idx16,
        "dinvc": dinvc, "dinv2c": dinv2c,
    }
    if has_bias:
        invd = np.zeros(NP, np.float32)
        invd[valid] = 1.0 / dinv_new[valid]
        common["invd"] = invd[None, :].astype(BF)
        bpad = biases[:3] + [np.pad(biases[3], (0, 128 - biases[3].size)),
                             biases[4]]
        common["brows"] = np.concatenate(bpad)[None, :].astype(BF)
    in_maps = []
    for b in range(B):
        m = dict(common)
        m["xT1"] = np.ascontiguousarray(vperm[b].T).astype(BF)
        m["hc1"] = hc[b][None, :].astype(BF)
        in_maps.append(m)
    meta = dict(NP=NP, T=T, C=C, perm=perm, valid=valid, B=B, N=N,
                FM=FM, F5=F5, FO=FO, has_bias=has_bias)
    return in_maps, meta


_BUILD_CACHE = {}


def run(inputs, trace=False):
    in_maps, meta = _prepare(**inputs)
    key = (meta["NP"], meta["C"], meta["FM"], meta["F5"], meta["FO"],
           meta["has_bias"])
    if key not in _BUILD_CACHE:
        t0 = time.time()
        _BUILD_CACHE[key] = _build_nc(meta["NP"], meta["T"], meta["C"],
                                      meta["FM"], meta["F5"], meta["FO"],
                                      meta["has_bias"])
        print(f"[kernel] built bass program in {time.time()-t0:.1f}s", file=sys.stderr)
    nc = _BUILD_CACHE[key]
    B = meta["B"]
    res = run_bass_kernel_spmd(nc, in_maps, core_ids=list(range(B)), trace=trace)
    perm, valid, N = meta["perm"], meta["valid"], meta["N"]
    out = np.empty((B, N, meta["FO"]), np.float32)
    for b in range(B):
        dev = res.results[b]["out"]
        out[b, perm[valid], :] = dev[valid, :]
    return out, res


def kernel(**inputs) -> np.ndarray:
    out, _ = run(inputs)
    return out
